# revision 10
# baseline (speedup 1.0000x reference)
"""Trainium2 Bass kernel for nn_BatchedGatedConvExperts.

Data-parallel over N across 8 cores (core k handles batch n=k).
Per core: depthwise 7x7 conv (E*C channels) -> GroupNorm(E groups) ->
cond affine -> grouped pw_in + SiLU gate -> grouped pw_out -> residual.

Flat-chunk quirk of the reference (torch .chunk on flat E*2C axis):
  silu input for output-expert e = pw_in block (e//2), rows (e%2)*96..+96,
  computed from y2 of expert e//2; gate half from block 4+e//2 / y2[4+e//2].
So experts are processed as pairs b in 0..3: y2[b], y2[4+b] -> outputs 2b, 2b+1.
"""
import sys

sys.path.insert(0, "/opt/trn_rl_repo")

import numpy as np

E, C, KS, CONDC = 8, 96, 7, 32
N, L, P = 8, 16, 16
PAD = KS // 2
S = L * P * P  # 4096
EC = E * C  # 768
EPS = 1e-5
PQ = P + 2 * PAD  # 22
NCHUNK = 512
NCH = S // NCHUNK  # 8

_BUILT = None


def _build():
    import concourse.bacc as bacc
    import concourse.mybir as mybir
    from concourse.masks import make_identity
    from concourse.tile import TileContext

    dt = mybir.dt
    f32 = dt.float32
    Alu = mybir.AluOpType
    Act = mybir.ActivationFunctionType

    nc = bacc.Bacc(None, target_bir_lowering=False)

    x_d = nc.declare_dram_parameter("x", [C, S], f32, isOutput=False)
    cond_d = nc.declare_dram_parameter("cond", [CONDC, S], f32, isOutput=False)
    dww_d = nc.declare_dram_parameter("dw_w", [EC, KS * KS], f32, isOutput=False)
    dwb_d = nc.declare_dram_parameter("dw_b", [EC], f32, isOutput=False)
    gnw_d = nc.declare_dram_parameter("gn_w", [EC], f32, isOutput=False)
    gnb_d = nc.declare_dram_parameter("gn_b", [EC], f32, isOutput=False)
    piw_d = nc.declare_dram_parameter("pw_in_w", [2 * EC, C], f32, isOutput=False)
    pib_d = nc.declare_dram_parameter("pw_in_b", [2 * EC], f32, isOutput=False)
    pow_d = nc.declare_dram_parameter("pw_out_w", [EC, C], f32, isOutput=False)
    pob_d = nc.declare_dram_parameter("pw_out_b", [EC], f32, isOutput=False)
    cw_d = nc.declare_dram_parameter("cond_w", [2 * EC, CONDC], f32, isOutput=False)
    cb_d = nc.declare_dram_parameter("cond_b", [2 * EC], f32, isOutput=False)
    out_d = nc.declare_dram_parameter("out", [EC, S], f32, isOutput=True)

    with TileContext(nc) as tc:
        with tc.tile_pool(name="wt", bufs=1) as wt, \
             tc.tile_pool(name="big", bufs=1) as big, \
             tc.tile_pool(name="y2p", bufs=3) as y2p, \
             tc.tile_pool(name="work", bufs=2) as work, \
             tc.tile_pool(name="small", bufs=4) as small, \
             tc.tile_pool(name="ps", bufs=4, space="PSUM") as ps, \
             tc.tile_pool(name="ps_s", bufs=2, space="PSUM") as ps_s:

            # ---------- weight prep ----------
            ident = wt.tile([128, 128], f32)
            make_identity(nc, ident)

            dw_w = wt.tile([C, E, KS * KS], f32)
            nc.sync.dma_start(out=dw_w, in_=dww_d[:].rearrange("(e c) t -> c e t", e=E))
            dw_b = wt.tile([C, E], f32)
            nc.sync.dma_start(out=dw_b, in_=dwb_d[:].rearrange("(e c) -> c e", e=E))
            gn_w = wt.tile([C, E], f32)
            nc.sync.dma_start(out=gn_w, in_=gnw_d[:].rearrange("(e c) -> c e", e=E))
            gn_b = wt.tile([C, E], f32)
            nc.sync.dma_start(out=gn_b, in_=gnb_d[:].rearrange("(e c) -> c e", e=E))
            cb_g = wt.tile([C, E], f32)
            nc.sync.dma_start(out=cb_g, in_=cb_d[:EC].rearrange("(e c) -> c e", e=E))
            cb_b = wt.tile([C, E], f32)
            nc.sync.dma_start(out=cb_b, in_=cb_d[EC:].rearrange("(e c) -> c e", e=E))

            lhsT_in = wt.tile([C + 1, 2 * EC], f32)
            lhsT_out = wt.tile([C + 1, EC], f32)
            lhsT_c = wt.tile([CONDC, 2 * EC], f32)
            for t in range(2 * EC // 128):  # 12
                w_raw = work.tile([128, C], f32, tag="wraw")
                nc.sync.dma_start(out=w_raw, in_=piw_d[t * 128:(t + 1) * 128, :])
                pt = ps.tile([C, 128], f32, tag="wtr", bufs=2)
                nc.tensor.transpose(pt, w_raw, ident)
                nc.vector.tensor_copy(lhsT_in[0:C, t * 128:(t + 1) * 128], pt)
            for t in range(EC // 128):  # 6
                w_raw = work.tile([128, C], f32, tag="wraw")
                nc.sync.dma_start(out=w_raw, in_=pow_d[t * 128:(t + 1) * 128, :])
                pt = ps.tile([C, 128], f32, tag="wtr", bufs=2)
                nc.tensor.transpose(pt, w_raw, ident)
                nc.vector.tensor_copy(lhsT_out[0:C, t * 128:(t + 1) * 128], pt)
            for t in range(2 * EC // 128):  # 12
                w_raw = work.tile([128, CONDC], f32, tag="wraw")
                nc.sync.dma_start(out=w_raw, in_=cw_d[t * 128:(t + 1) * 128, :])
                pt = ps.tile([CONDC, 128], f32, tag="wtr", bufs=2)
                nc.tensor.transpose(pt, w_raw, ident)
                nc.vector.tensor_copy(lhsT_c[:, t * 128:(t + 1) * 128], pt)
            nc.sync.dma_start(out=lhsT_in[C:C + 1, :], in_=pib_d[:])
            nc.sync.dma_start(out=lhsT_out[C:C + 1, :], in_=pob_d[:])

            # ---------- x / cond load ----------
            cond_sb = big.tile([CONDC, S], f32)
            nc.sync.dma_start(out=cond_sb, in_=cond_d[:])

            x_pad = big.tile([C, L, PQ, PQ], f32)
            nc.vector.memset(x_pad, 0.0)
            for l in range(L):
                nc.sync.dma_start(
                    out=x_pad[:, l, PAD:PAD + P, PAD:PAD + P],
                    in_=x_d[:, l * P * P:(l + 1) * P * P])
            x_in = x_pad[:, :, PAD:PAD + P, PAD:PAD + P]  # residual view

            ones96 = wt.tile([C, 1], f32)
            nc.vector.memset(ones96, 1.0)
            ones_row = wt.tile([1, C], f32)
            nc.vector.memset(ones_row, 1.0)
            eps11 = wt.tile([1, 1], f32)
            nc.vector.memset(eps11, EPS)

            def build_y2(e):
                """dw conv + GN + cond affine for expert e -> y2 [97, S]."""
                acc = work.tile([C, S], f32, tag="acc", name="acc")
                t = 0
                for i in range(KS):
                    for j in range(KS):
                        xs = x_pad[:, :, i:i + P, j:j + P]
                        if t == 0:
                            nc.vector.tensor_scalar(
                                acc, xs, dw_w[:, e, 0:1], dw_b[:, e:e + 1],
                                Alu.mult, Alu.add)
                        else:
                            tmp = work.tile([C, S], f32, tag="dwtmp", bufs=1,
                                            name="dwtmp")
                            nc.vector.tensor_scalar_mul(tmp, xs, dw_w[:, e, t:t + 1])
                            nc.vector.tensor_tensor(acc, acc, tmp, Alu.add)
                        t += 1

                # GN stats via bn_stats/bn_aggr then partition-reduce matmul
                stats = small.tile([C, NCH, nc.vector.BN_STATS_DIM], f32,
                                   tag="stats", name="stats")
                for sc in range(NCH):
                    nc.vector.bn_stats(
                        out=stats[:, sc, :],
                        in_=acc[:, sc * NCHUNK:(sc + 1) * NCHUNK])
                mv = small.tile([C, nc.vector.BN_AGGR_DIM], f32, tag="mv",
                                name="mv")
                nc.vector.bn_aggr(out=mv, in_=stats)
                st3 = small.tile([C, 3], f32, tag="st3", name="st3")
                nc.vector.tensor_copy(st3[:, 0:2], mv)
                nc.vector.tensor_tensor(st3[:, 2:3], mv[:, 0:1], mv[:, 0:1],
                                        Alu.mult)
                ps_stat = ps_s.tile([1, 3], f32, tag="pstat", name="pstat", bufs=1)
                nc.tensor.matmul(ps_stat, ones96, st3, start=True, stop=True)

                # mean = S0/96 ; Ex2 = (S1+S2)/96 ; var = Ex2 - mean^2
                st_sb = small.tile([1, 3], f32, tag="st_sb", name="st_sb")
                nc.vector.tensor_copy(st_sb, ps_stat)
                mean11 = small.tile([1, 1], f32, tag="mean11", name="mean11")
                nc.vector.tensor_scalar_mul(mean11, st_sb[0:1, 0:1], 1.0 / C)
                ex2 = small.tile([1, 1], f32, tag="ex2", name="ex2")
                nc.vector.tensor_tensor(ex2, st_sb[0:1, 1:2], st_sb[0:1, 2:3],
                                        Alu.add)
                var11 = small.tile([1, 1], f32, tag="var11", name="var11")
                nc.vector.tensor_scalar_mul(var11, ex2, 1.0 / C)
                msq11 = small.tile([1, 1], f32, tag="msq11", name="msq11")
                nc.vector.tensor_tensor(msq11, mean11, mean11, Alu.mult)
                nc.vector.tensor_tensor(var11, var11, msq11, Alu.subtract)
                std11 = small.tile([1, 1], f32, tag="std11", name="std11")
                nc.scalar.activation(std11, var11, Act.Sqrt, bias=eps11[0:1, 0:1])
                rstd11 = small.tile([1, 1], f32, tag="rstd11", name="rstd11")
                nc.vector.reciprocal(rstd11, std11)
                mr = small.tile([1, 2], f32, tag="mr", name="mr")
                nc.vector.tensor_copy(mr[:, 0:1], mean11)
                nc.vector.tensor_copy(mr[:, 1:2], rstd11)
                # broadcast [1,2] -> [96,2] via rank-1 matmul: ones_row.T @ mr
                bc = ps_s.tile([C, 2], f32, tag="bc", name="bc", bufs=1)
                nc.tensor.matmul(bc, ones_row, mr, start=True, stop=True)

                a_vec = small.tile([C, 1], f32, tag="a_vec", name="a_vec")
                nc.vector.tensor_tensor(a_vec, gn_w[:, e:e + 1], bc[:, 1:2],
                                        Alu.mult)
                mb = small.tile([C, 1], f32, tag="mb", name="mb")
                nc.vector.tensor_tensor(mb, bc[:, 0:1], a_vec, Alu.mult)
                b_vec = small.tile([C, 1], f32, tag="b_vec", name="b_vec")
                nc.vector.tensor_tensor(b_vec, gn_b[:, e:e + 1], mb, Alu.subtract)

                y2 = y2p.tile([C + 1, S], f32, tag="y2", name="y2")
                nc.vector.memset(y2[C:C + 1, :], 1.0)
                nc.scalar.activation(y2[0:C, :], acc, Act.Identity,
                                     bias=b_vec, scale=a_vec)

                # cond affine applied per spatial chunk
                for sc in range(NCH):
                    sl = slice(sc * NCHUNK, (sc + 1) * NCHUNK)
                    pg = ps.tile([C, NCHUNK], f32, tag="mm", name="pg")
                    nc.tensor.matmul(pg, lhsT_c[:, e * C:(e + 1) * C],
                                     cond_sb[:, sl], start=True, stop=True)
                    gam = small.tile([C, NCHUNK], f32, tag="gam", name="gam")
                    nc.vector.tensor_scalar(gam, pg, cb_g[:, e:e + 1], 1.0,
                                            Alu.add, Alu.add)
                    pb = ps.tile([C, NCHUNK], f32, tag="mm", name="pb")
                    nc.tensor.matmul(pb, lhsT_c[:, EC + e * C:EC + (e + 1) * C],
                                     cond_sb[:, sl], start=True, stop=True)
                    bet = small.tile([C, NCHUNK], f32, tag="bet", name="bet")
                    nc.vector.tensor_scalar(bet, pb, cb_b[:, e:e + 1], None,
                                            Alu.add)
                    nc.vector.tensor_tensor(y2[0:C, sl], y2[0:C, sl], gam,
                                            Alu.mult)
                    nc.vector.tensor_tensor(y2[0:C, sl], y2[0:C, sl], bet,
                                            Alu.add)
                return y2

            for b in range(E // 2):
                y2_lin = build_y2(b)
                y2_gate = build_y2(4 + b)
                for half in range(2):  # output experts 2b, 2b+1
                    e = 2 * b + half
                    for sc in range(NCH):
                        sl = slice(sc * NCHUNK, (sc + 1) * NCHUNK)
                        pl = ps.tile([C, NCHUNK], f32, tag="mm", name="pl")
                        nc.tensor.matmul(
                            pl, lhsT_in[:, b * 2 * C + half * C:
                                        b * 2 * C + (half + 1) * C],
                            y2_lin[:, sl], start=True, stop=True)
                        pgt = ps.tile([C, NCHUNK], f32, tag="mm", name="pgt")
                        nc.tensor.matmul(
                            pgt, lhsT_in[:, (4 + b) * 2 * C + half * C:
                                         (4 + b) * 2 * C + (half + 1) * C],
                            y2_gate[:, sl], start=True, stop=True)
                        sil = small.tile([C, NCHUNK], f32, tag="sil", name="sil")
                        nc.scalar.activation(sil, pl, Act.Silu)
                        gt = small.tile([C + 1, NCHUNK], f32, tag="gt", name="gt")
                        nc.vector.memset(gt[C:C + 1, :], 1.0)
                        nc.vector.tensor_tensor(gt[0:C, :], sil, pgt, Alu.mult)

                        po = ps.tile([C, NCHUNK], f32, tag="mm", name="po")
                        nc.tensor.matmul(po, lhsT_out[:, e * C:(e + 1) * C],
                                         gt, start=True, stop=True)
                        y3c = small.tile([C, NCHUNK], f32, tag="y3c", name="y3c")
                        lpc = NCHUNK // (P * P)  # l-planes per chunk
                        xv = x_pad[:, sc * lpc:(sc + 1) * lpc,
                                   PAD:PAD + P, PAD:PAD + P]
                        nc.vector.tensor_tensor(
                            y3c.rearrange("c (a b d) -> c a b d", a=lpc, b=P),
                            po.rearrange("c (a b d) -> c a b d", a=lpc, b=P),
                            xv, Alu.add)
                        nc.sync.dma_start(out=out_d[e * C:(e + 1) * C, sl],
                                          in_=y3c)

    nc.finalize()
    return nc


def _get_built():
    global _BUILT
    if _BUILT is None:
        _BUILT = _build()
    return _BUILT


def kernel(**inputs):
    from concourse.bass_utils import run_bass_kernel_spmd

    nc = _get_built()
    x = np.asarray(inputs["x"], dtype=np.float32)
    cond = np.asarray(inputs["cond"], dtype=np.float32)
    base = {
        "dw_w": np.asarray(inputs["dw_weight"], np.float32).reshape(EC, KS * KS),
        "dw_b": np.asarray(inputs["dw_bias"], np.float32),
        "gn_w": np.asarray(inputs["gn_weight"], np.float32),
        "gn_b": np.asarray(inputs["gn_bias"], np.float32),
        "pw_in_w": np.asarray(inputs["pw_in_weight"], np.float32),
        "pw_in_b": np.asarray(inputs["pw_in_bias"], np.float32),
        "pw_out_w": np.asarray(inputs["pw_out_weight"], np.float32),
        "pw_out_b": np.asarray(inputs["pw_out_bias"], np.float32),
        "cond_w": np.asarray(inputs["cond_w"], np.float32),
        "cond_b": np.asarray(inputs["cond_b"], np.float32),
    }
    in_maps = []
    for k in range(N):
        m = dict(base)
        m["x"] = np.ascontiguousarray(x[k].reshape(C, S))
        m["cond"] = np.ascontiguousarray(cond[k].reshape(CONDC, S))
        in_maps.append(m)
    res = run_bass_kernel_spmd(nc, in_maps, list(range(N)))
    out = np.empty((N, E, C, L, P, P), dtype=np.float32)
    for k in range(N):
        out[k] = res.results[k]["out"].reshape(E, C, L, P, P)
    return out
